# revision 44
# baseline (speedup 1.0000x reference)
"""Trainium2 Bass kernel for nn_CenMoEDynamicsModel (MoE routing).

Contract: kernel(**inputs) takes FULL unsharded numpy inputs and returns the
FULL [64, 2048, 128] f32 output. Internally: data-parallel over B across 8
NeuronCores (8 batches/core), expert weights replicated, run via
run_bass_kernel_spmd.

Math (per batch b):
  x = [z|a]                       [N, D]     D = 192
  w = x @ phi                     [N, E]     E = 16
  dispatch = softmax_n(w); xin = dispatch^T @ x          [E, D]
  h = mish(LN(xin@W1+b1)); h = mish(LN(h@W2+b2)); EO = h@W3+b3   [E, DZ]
  combine = softmax_e(w); out = combine @ EO             [N, DZ]

Key tricks (all softmaxes via unshifted exp — logits are O(5) so exp is safe):
  - ones column appended to x tiles => dispatch denominator s_e appears as an
    extra output row of the xin matmul (no partition reductions).
  - L1 computed on unnormalized xin with b1 folded in as an augmented weight
    row against the s_e row; scaling L1 rows by r=1/s then normalizes both
    the dispatch AND the bias exactly.
  - MLP activations packed [(e,b), H] across partitions => full-width LN/Mish.
    Since PE outputs must start at base partition 0/32/64/96, each MLP layer
    writes its output TRANSPOSED ([h, (e,b)] — free-dim offsets are
    unconstrained), which is then PE-transposed back into the dense pack.
  - per-batch e-major data (exp(w), expert outputs) lives in per-batch
    16-partition tiles at base partition 0 (PE tile_position offsets are
    rejected for float32r matmuls, so everything stays at (0, 0)).
  - combine uses exp(w) directly as lhsT (e-major) with a ones column in the
    EO rhs producing the softmax denominator as output column DZ.
"""

import sys

import numpy as np

sys.path.insert(0, "/opt/trn_rl_repo")

from contextlib import ExitStack

import concourse.bass as bass
import concourse.tile as tile
from concourse import masks, mybir

F32 = mybir.dt.float32
R32 = mybir.dt.float32r
AF = mybir.ActivationFunctionType

# PE runs fp32 matmuls at 4 cycles/row but float32r (same 4-byte layout,
# reduced-precision multiply) at 1 cycle/row when the output is >=256 elems.
USE_F32R = True


def _r(ap):
    return ap.bitcast(R32) if USE_F32R else ap

LN_EPS = 1e-5
NCORES = 8


def _split_drain_and_barrier(self, tick_clock, wait_clock):
    """Replacement for TileContext._drain_and_barrier.

    The stock version attaches every outstanding semaphore wait to ONE tail
    Drain instruction; this walrus build's codegen rejects Drains with more
    than a couple of sync waits ("Too many sync wait commands"). Emit one
    single-wait Drain per logical proc instead (the SP queue executes them in
    order, so the final bare drain still happens after everything finished).
    """
    from concourse.vector_clock import ScopedClock, VectorClock

    nc = self.nc
    gc = tick_clock.global_clock
    n = len(gc)
    for i in range(n):
        t = gc[i]
        if t <= 0:
            continue
        v = VectorClock([0] * n)
        v.require_at_least(i, t)
        d = nc.sync.drain()
        wait_clock.add_sem_waits(d.ins, ScopedClock({None: v}))
    nc.sync.drain()
    nc.all_engine_barrier()
    assert self.sems is not None
    popped = nc._tile_sem_poison_stack.pop()
    assert popped is self._sem_poison
    nc.clear_and_free_semaphores(list(self.sems.allocated().values()))
    nc.all_engine_barrier()


tile.TileContext._drain_and_barrier = _split_drain_and_barrier

# This walrus build rejects instructions carrying more than a couple of sync
# waits ("Too many sync wait commands" in CoreV3 codegen), while Tile freely
# attaches 3+. Split excess waits onto NoOp carrier instructions (same engine
# queue, executed in order => semantics preserved) at BIR-serialization time.
_MAX_WAITS = 1


def _split_waits_json(bir: bytes) -> bytes:
    import orjson

    m = orjson.loads(bir)
    changed = False
    ctr = 0
    for f in m.get("functions", []):
        for b in f.get("blocks", []):
            out = []
            for i in b.get("instructions", []):
                si = i.get("sync_info")
                ow = (si or {}).get("on_wait") or []
                if len(ow) > _MAX_WAITS:
                    head = ow[: -_MAX_WAITS]
                    for j in range(0, len(head), _MAX_WAITS):
                        ctr += 1
                        out.append(
                            {
                                "debug": i.get("debug", 0),
                                "engine": i["engine"],
                                "ins": [],
                                "outs": [],
                                "name": f"{i['name']}-wsplit{ctr}",
                                "opcode": "NoOp",
                                "sync_info": {
                                    "on_wait": head[j : j + _MAX_WAITS],
                                    "on_update": [],
                                },
                            }
                        )
                    si["on_wait"] = ow[-_MAX_WAITS:]
                    changed = True
                out.append(i)
            b["instructions"] = out
    return orjson.dumps(m) if changed else bir


_orig_to_json_bytes = bass.Bass.to_json_bytes


def _patched_to_json_bytes(self):
    return _split_waits_json(_orig_to_json_bytes(self))


bass.Bass.to_json_bytes = _patched_to_json_bytes


def build_nc(BC, N, DZ, DA, E, H1, H2, has_b2, has_b3, has_g1, has_g2):
    """Build the per-core Bass program.

    BC batches per core. Requires DZ == 128, N % 512 == 0, H1 % 128 == 0,
    H2 % 128 == 0, E * BC <= 128, E <= 16.
    """
    D = DZ + DA
    NT = N // 128
    NC = N // 512
    EB = E * BC
    C1 = H1 // 128
    C2 = H2 // 128
    assert DZ == 128 and DA < 128 and E * BC <= 128 and N % 512 == 0 and E <= 16

    nc = bass.Bass()

    def mmr(out, lhsT, rhs, **kw):
        return nc.tensor.matmul(out, _r(lhsT), _r(rhs), **kw)

    def trr(out, in_, ident_ap, **kw):
        return nc.tensor.transpose(_r(out), _r(in_), _r(ident_ap), **kw)

    z = nc.dram_tensor("z", [BC, N, DZ], R32, kind="ExternalInput")
    a = nc.dram_tensor("a", [BC, N, DA], R32, kind="ExternalInput")
    # host-pretransposed copies (features-major) — saves 2 PE transposes + 2
    # PSUM evacuation copies per token tile
    zT = nc.dram_tensor("zT", [BC, DZ, N], R32, kind="ExternalInput")
    aT = nc.dram_tensor("aT", [BC, DA, N], R32, kind="ExternalInput")
    phi = nc.dram_tensor("phi", [D, E], R32, kind="ExternalInput")
    # w1 is bias-augmented: row D holds b1[e]
    w1 = nc.dram_tensor("w1", [E, D + 1, H1], R32, kind="ExternalInput")
    w2 = nc.dram_tensor("w2", [E, H1, H2], R32, kind="ExternalInput")
    w3 = nc.dram_tensor("w3", [E, H2, DZ], R32, kind="ExternalInput")
    if has_b2:
        b2 = nc.dram_tensor("b2", [E, H2], R32, kind="ExternalInput")
    if has_b3:
        b3 = nc.dram_tensor("b3", [E, DZ], R32, kind="ExternalInput")
    if has_g1:
        g1r = nc.dram_tensor("g1r", [EB, H1], F32, kind="ExternalInput")
        be1r = nc.dram_tensor("be1r", [EB, H1], F32, kind="ExternalInput")
    if has_g2:
        g2r = nc.dram_tensor("g2r", [EB, H2], F32, kind="ExternalInput")
        be2r = nc.dram_tensor("be2r", [EB, H2], F32, kind="ExternalInput")
    ident_d = nc.dram_tensor("ident_d", [128, 128], R32, kind="ExternalInput")
    ones_d = nc.dram_tensor("ones_d", [128, 128], R32, kind="ExternalInput")
    out = nc.dram_tensor("out", [BC, N, DZ], F32, kind="ExternalOutput")

    with tile.TileContext(nc) as tc, ExitStack() as ctx:
        perm = ctx.enter_context(tc.tile_pool(name="perm", bufs=1))
        # identity / ones constants come pre-built from DRAM: several engines
        # cannot write float32r, but a same-dtype DMA can.
        ident = perm.tile([128, 128], R32)
        nc.sync.dma_start(ident[:], ident_d[:, :])
        ones_sb = perm.tile([128, 128], R32)
        nc.sync.dma_start(ones_sb[:], ones_d[:, :])
        phi_hi = perm.tile([128, E], R32)
        phi_lo = perm.tile([DA, E], R32)
        nc.sync.dma_start(phi_hi[:], phi[0:128, :])
        nc.sync.dma_start(phi_lo[:], phi[128:D, :])
        # exp(weights), e-major per batch: [E, N] at base partition 0
        expCT = [
            perm.tile([E, N], R32, tag=f"expCT{b}", name=f"expCT{b}")
            for b in range(BC)
        ]
        # unnormalized xin^T: col e*BC+b; xin_lo row DA = s_e (softmax denom)
        xin_hi = perm.tile([128, EB], R32)
        xin_lo = perm.tile([DA + 1, EB], R32)
        # expert outputs per batch: [E, DZ+1] at base partition 0; col DZ = ones
        # DZ+2 not DZ+1: float32r matmuls need an even moving-dim size
        eo = [
            perm.tile([E, DZ + 2], R32, tag=f"eo{b}", name=f"eo{b}")
            for b in range(BC)
        ]
        r_row = perm.tile([1, EB], F32)
        r_col = perm.tile([EB, 1], F32)
        ones1 = ones_sb
        eps_col = perm.tile([128, 1], F32)
        nc.vector.memset(eps_col[:], LN_EPS)

        # ---------------- Phase A: routing + xin, per batch ----------------
        with tc.tile_pool(name="pa", bufs=2) as pa, tc.tile_pool(
            name="pa_ec", bufs=3
        ) as pa_ec, tc.tile_pool(name="pa_ps", bufs=2, space="PSUM") as pps, tc.tile_pool(
            name="pa_ps_tr", bufs=2, space="PSUM"
        ) as ppst:
            for b in range(BC):
                # x native (tokens on partitions) with two ones columns (one
                # for the softmax-denominator trick, one to keep the f32r
                # moving size even)
                x_sb = pa.tile([128, NT * (D + 2)], R32, tag="x")
                xv = x_sb[:].rearrange("p (t c) -> p t c", c=D + 2)
                nc.scalar.copy(
                    xv[:, :, D : D + 2],
                    ones_sb[:].rearrange("p (t k) -> p t k", k=2)[:, 0:NT, :],
                )
                nc.sync.dma_start(
                    xv[:, :, 0:DZ], z[b].rearrange("(p t) d -> p t d", t=NT)
                )
                nc.sync.dma_start(
                    xv[:, :, DZ:D], a[b].rearrange("(p t) d -> p t d", t=NT)
                )
                # x transposed (features on partitions) straight from DRAM
                xT_hi = pa.tile([128, N], R32, tag="xth")
                xT_lo = pa.tile([DA, N], R32, tag="xtl")
                nc.sync.dma_start(xT_hi[:], zT[b])
                nc.sync.dma_start(xT_lo[:], aT[b])
                # weightsT = phi^T @ xT, then exp -> expCT[b]
                for c in range(NC):
                    wps = pps.tile([E, 512], F32, tag="wt")
                    sl = slice(512 * c, 512 * (c + 1))
                    mmr(wps[:], phi_hi[:], xT_hi[:, sl], start=True, stop=False)
                    mmr(wps[:], phi_lo[:], xT_lo[:, sl], start=False, stop=True)
                    nc.scalar.activation(expCT[b][:, sl], wps[:], AF.Exp)
                # xin = expC^T @ x_aug, e-major [E, D+2]; col D holds s_e
                xps = pps.tile([E, D + 2], F32, tag="xin")
                ecv = expCT[b][:].rearrange("e (p t) -> e t p", t=NT)
                for t in range(NT):
                    pt = ppst.tile([128, 128], F32, tag="tr")
                    trr(
                        pt[0:128, 0:E],
                        ecv[:, t, :],
                        ident[0:E, 0:E],
                    )
                    ec = pa_ec.tile([128, E], R32, tag="ec")
                    nc.vector.tensor_copy(ec[:], pt[0:128, 0:E].bitcast(R32))
                    mmr(
                        xps[:],
                        ec[:],
                        xv[:, t, :],
                        start=(t == 0),
                        stop=(t == NT - 1),
                    )
                # transpose xin into the packed [d, (e, b)] layout for L1
                xin_sb = pa_ec.tile([E, D + 2], R32, tag="xin_sb")
                nc.scalar.copy(xin_sb[:], xps[:])
                xhv = xin_hi[:].rearrange("p (e b) -> p e b", b=BC)
                xlv = xin_lo[:].rearrange("p (e b) -> p e b", b=BC)
                pth = ppst.tile([128, 128], F32, tag="tr")
                trr(pth[0:128, 0:E], xin_sb[:, 0:128], ident[0:E, 0:E])
                nc.vector.tensor_copy(xhv[:, :, b], pth[0:128, 0:E].bitcast(R32))
                ptl = ppst.tile([128, 128], F32, tag="tr")
                trr(ptl[0 : DA + 1, 0:E], xin_sb[:, 128 : D + 1], ident[0:E, 0:E])
                nc.scalar.copy(xlv[:, :, b], ptl[0 : DA + 1, 0:E].bitcast(R32))

        # ---------------- MLP phase (packed over (e, b) rows) ----------------
        nc.vector.reciprocal(r_row[:], xin_lo[DA : DA + 1, :].bitcast(F32))
        nc.sync.dma_start(r_col[:], r_row[:])

        def ln_mish(hs, pool, H, gr, ber):
            """LayerNorm + mish of SBUF [EB, H] (g/be general path optional)."""
            s1 = pool.tile([EB, 1], F32, tag="s1")
            nc.vector.reduce_sum(s1[:], hs, axis=mybir.AxisListType.X)
            mean = pool.tile([EB, 1], F32, tag="mean")
            nc.scalar.mul(mean[:], s1[:], 1.0 / H)
            xc = pool.tile([EB, H], F32, tag="xc")
            nc.vector.tensor_scalar_sub(xc[:], hs, mean[:])
            sq = pool.tile([EB, H], F32, tag="sq")
            var = pool.tile([EB, 1], F32, tag="var")
            nc.scalar.activation(sq[:], xc[:], AF.Square, accum_out=var[:])
            std = pool.tile([EB, 1], F32, tag="std")
            nc.scalar.activation(
                std[:], var[:], AF.Sqrt, bias=eps_col[0:EB, :], scale=1.0 / H
            )
            rstd = pool.tile([EB, 1], F32, tag="rstd")
            nc.vector.reciprocal(rstd[:], std[:])
            xn = pool.tile([EB, H], F32, tag="xn")
            nc.vector.tensor_scalar_mul(xn[:], xc[:], rstd[:])
            if gr is not None:
                xg = pool.tile([EB, H], F32, tag="xg")
                nc.vector.tensor_mul(xg[:], xn[:], gr)
                xn = pool.tile([EB, H], F32, tag="xb")
                nc.vector.tensor_add(xn[:], xg[:], ber)
            # mish(x) = x * tanh(ln(1 + e^x))  (Mish table unsupported in sim)
            ex = pool.tile([EB, H], F32, tag="ex")
            nc.scalar.activation(ex[:], xn[:], AF.Exp)
            sp = pool.tile([EB, H], F32, tag="sp")
            nc.scalar.activation(sp[:], ex[:], AF.Ln, bias=1.0)
            th = pool.tile([EB, H], F32, tag="th")
            nc.scalar.activation(th[:], sp[:], AF.Tanh)
            hm = pool.tile([EB, H], R32, tag="hm")
            nc.vector.tensor_mul(hm[:], xn[:], th[:])
            return hm

        def transpose_pack(hm, pool, ppool, H, name):
            """[EB, H] -> hT [128, (H//128)*EB], chunk c at cols [c*EB,(c+1)*EB)."""
            hT = pool.tile([128, (H // 128) * EB], R32, tag=name, name=name)
            for c in range(H // 128):
                pt = ppool.tile([128, 128], F32, tag="mtr")
                trr(
                    pt[:, 0:EB],
                    hm[:, 128 * c : 128 * (c + 1)],
                    ident[0:EB, 0:EB],
                )
                if c % 2 == 0:
                    nc.vector.tensor_copy(
                        hT[:, c * EB : (c + 1) * EB], pt[:, 0:EB].bitcast(R32)
                    )
                else:
                    nc.scalar.copy(
                        hT[:, c * EB : (c + 1) * EB], pt[:, 0:EB].bitcast(R32)
                    )
            return hT

        with tc.tile_pool(name="pw", bufs=4) as pw, tc.tile_pool(
            name="pm", bufs=1
        ) as pm, tc.tile_pool(name="pm_st", bufs=3) as pst, tc.tile_pool(
            name="pm_ps", bufs=3, space="PSUM"
        ) as pmps, tc.tile_pool(
            name="pm_ps_tr", bufs=2, space="PSUM"
        ) as pmpst, tc.tile_pool(name="pm_ps_eo", bufs=1, space="PSUM") as pmpse:
            if has_b2:
                b2sb = pm.tile([1, E * H2], R32, tag="b2sb")
                nc.sync.dma_start(b2sb[:], b2.rearrange("e h -> (e h)")[None, :])
            if has_b3:
                b3sb = pm.tile([1, E * DZ], R32, tag="b3sb")
                nc.sync.dma_start(b3sb[:], b3.rearrange("e h -> (e h)")[None, :])
            g1sb = be1sb = g2sb = be2sb = None
            if has_g1:
                g1sb = pm.tile([EB, H1], F32, tag="g1sb")
                be1sb = pm.tile([EB, H1], F32, tag="be1sb")
                nc.sync.dma_start(g1sb[:], g1r[:, :])
                nc.sync.dma_start(be1sb[:], be1r[:, :])
            if has_g2:
                g2sb = pm.tile([EB, H2], F32, tag="g2sb")
                be2sb = pm.tile([EB, H2], F32, tag="be2sb")
                nc.sync.dma_start(g2sb[:], g2r[:, :])
                nc.sync.dma_start(be2sb[:], be2r[:, :])

            # Layer 1: bias folded via augmented row (lhsT row DA of xin_lo = s);
            # scaling rows by r = 1/s normalizes dispatch and bias exactly.
            # Activation-stationary: lhsT = xin [d, 8] (tiny weight loads),
            # rhs = W1[e]. Per-expert [8, H] PSUM rows are packed into the
            # dense [EB, H] SBUF tensor via copy + partition-moving DMA.
            h1_all = pm.tile([EB, H1], F32, tag="h1_all", name="h1_all")
            for e in range(E):
                w1h = pw.tile([128, H1], R32, tag="w1h")
                nc.gpsimd.dma_start(w1h[:], w1[e, 0:128, :])
                w1l = pw.tile([D + 1 - 128, H1], R32, tag="w1l")
                nc.gpsimd.dma_start(w1l[:], w1[e, 128 : D + 1, :])
                hp = pmps.tile([BC, H1], F32, tag="hp")
                mmr(
                    hp[:],
                    xin_hi[:, e * BC : (e + 1) * BC],
                    w1h[:],
                    start=True,
                    stop=False,
                )
                mmr(
                    hp[:],
                    xin_lo[:, e * BC : (e + 1) * BC],
                    w1l[:],
                    start=False,
                    stop=True,
                )
                hst = pst.tile([BC, H1], F32, tag="hst")
                if e % 2 == 0:
                    nc.vector.tensor_copy(hst[:], hp[:])
                else:
                    nc.scalar.copy(hst[:], hp[:])
                nc.gpsimd.dma_start(h1_all[e * BC : (e + 1) * BC, :], hst[:])
            h1s = pm.tile([EB, H1], F32, tag="h1s")
            nc.vector.tensor_scalar_mul(h1s[:], h1_all[:], r_col[:])
            h1m = ln_mish(
                h1s[:],
                pm,
                H1,
                g1sb[:] if has_g1 else None,
                be1sb[:] if has_g1 else None,
            )
            h1T = transpose_pack(h1m, pm, pmpst, H1, "h1T")

            # Layer 2 (activation-stationary, like L1)
            h2_all = pm.tile([EB, H2], F32, tag="h2_all", name="h2_all")
            for e in range(E):
                w2t = pw.tile([128, C1 * H2], R32, tag="w2t")
                w2v = w2t[:].rearrange("p (c h) -> p c h", c=C1)
                nc.gpsimd.dma_start(
                    w2v, w2[e].rearrange("(c p) h -> p c h", p=128)
                )
                hp = pmps.tile([BC, H2], F32, tag="hp")
                for c1 in range(C1):
                    mmr(
                        hp[:],
                        h1T[:, c1 * EB + e * BC : c1 * EB + (e + 1) * BC],
                        w2v[:, c1, :],
                        start=(c1 == 0),
                        stop=(c1 == C1 - 1 and not has_b2),
                    )
                if has_b2:
                    mmr(
                        hp[:],
                        ones1[0:1, 0:BC],
                        b2sb[0:1, e * H2 : (e + 1) * H2],
                        start=False,
                        stop=True,
                    )
                hst = pst.tile([BC, H2], F32, tag="hst")
                if e % 2 == 0:
                    nc.vector.tensor_copy(hst[:], hp[:])
                else:
                    nc.scalar.copy(hst[:], hp[:])
                nc.gpsimd.dma_start(h2_all[e * BC : (e + 1) * BC, :], hst[:])
            h2m = ln_mish(
                h2_all[:],
                pm,
                H2,
                g2sb[:] if has_g2 else None,
                be2sb[:] if has_g2 else None,
            )
            h2T = transpose_pack(h2m, pm, pmpst, H2, "h2T")

            # Layer 3 -> EOT [DZ, (b, e)] col b*E+e (free-dim strided writes)
            eops = pmpse.tile([128, EB], F32, tag="eot")
            for e in range(E):
                w3t = pw.tile([128, C2 * DZ], R32, tag="w3t")
                nc.gpsimd.dma_start(
                    w3t[:].rearrange("p (c d) -> p c d", c=C2),
                    w3[e].rearrange("(c p) d -> p c d", p=128),
                )
                for c in range(C2):
                    mmr(
                        eops[0:DZ, e * BC : (e + 1) * BC],
                        w3t[:, c * DZ : (c + 1) * DZ],
                        h2T[:, c * EB + e * BC : c * EB + (e + 1) * BC],
                        start=(c == 0),
                        stop=(c == C2 - 1 and not has_b3),
                    )
                if has_b3:
                    mmr(
                        eops[0:DZ, e * BC : (e + 1) * BC],
                        b3sb[0:1, e * DZ : (e + 1) * DZ],
                        ones1[0:1, 0:BC],
                        start=False,
                        stop=True,
                    )
            eot_sb = pm.tile([128, EB], R32, tag="eot_sb")
            nc.vector.tensor_copy(
                eot_sb[0:DZ, :].rearrange("p (b e) -> p b e", e=E),
                eops[0:DZ, :].rearrange("p (e b) -> p b e", b=BC),
            )
            for b in range(BC):
                nc.scalar.copy(eo[b][:, DZ : DZ + 2], ones_sb[0:E, 0:2])
                pt = pmpst.tile([128, 128], F32, tag="mtr")
                trr(
                    pt[0:E, 0:DZ],
                    eot_sb[0:DZ, b * E : (b + 1) * E],
                    ident[0:DZ, 0:DZ],
                )
                nc.vector.tensor_copy(eo[b][:, 0:DZ], pt[0:E, 0:DZ].bitcast(R32))

        # ---------------- Combine phase ----------------
        with tc.tile_pool(name="pc", bufs=4) as pc, tc.tile_pool(
            name="pc_st", bufs=2
        ) as pcst, tc.tile_pool(name="pc_ps", bufs=4, space="PSUM") as pcps:
            for b in range(BC):
                # per-batch output staging => one store DMA per batch (the
                # sync sequencer's DIRECT2D dispatch is ~0.8us per dma_start)
                osb = pcst.tile([128, NT * DZ], F32, tag="osb")
                ov = osb[:].rearrange("p (t d) -> p t d", d=DZ)
                ecv = expCT[b][:].rearrange("e (p t) -> e t p", t=NT)
                for t in range(NT):
                    ops = pcps.tile([128, DZ + 2], F32, tag="o")
                    mmr(
                        ops[:],
                        ecv[:, t, :],
                        eo[b][:, :],
                        start=True,
                        stop=True,
                    )
                    rn = pc.tile([128, 1], F32, tag="rn")
                    nc.vector.reciprocal(rn[:], ops[:, DZ : DZ + 1])
                    if t % 2 == 0:
                        nc.scalar.mul(ov[:, t, :], ops[:, 0:DZ], rn[:])
                    else:
                        nc.vector.tensor_scalar_mul(ov[:, t, :], ops[:, 0:DZ], rn[:])
                nc.sync.dma_start(
                    out[b].rearrange("(p t) d -> p t d", t=NT), ov[:, :, :]
                )
    return nc


# ---------------------------------------------------------------------------
# Host wrapper
# ---------------------------------------------------------------------------

_CACHE = {}


def _get_nc(key, *args):
    if key not in _CACHE:
        _CACHE[key] = build_nc(*args)
    return _CACHE[key]


def _prepare(z, a, phi, W1, b1, g1, be1, W2, b2, g2, be2, W3, b3):
    """Build (cached) the Bass program and per-core input maps."""
    z = np.asarray(z, np.float32)
    a = np.asarray(a, np.float32)
    phi = np.asarray(phi, np.float32)
    W1 = np.asarray(W1, np.float32)
    b1 = np.asarray(b1, np.float32)
    g1 = np.asarray(g1, np.float32)
    be1 = np.asarray(be1, np.float32)
    W2 = np.asarray(W2, np.float32)
    b2 = np.asarray(b2, np.float32)
    g2 = np.asarray(g2, np.float32)
    be2 = np.asarray(be2, np.float32)
    W3 = np.asarray(W3, np.float32)
    b3 = np.asarray(b3, np.float32)

    B, N, DZ = z.shape
    DA = a.shape[2]
    E = W1.shape[0]
    H1 = W1.shape[2]
    H2 = W2.shape[2]
    BC = B // NCORES

    has_b2 = bool(np.any(b2))
    has_b3 = bool(np.any(b3))
    has_g1 = not (np.all(g1 == 1.0) and np.all(be1 == 0.0))
    has_g2 = not (np.all(g2 == 1.0) and np.all(be2 == 0.0))

    key = (BC, N, DZ, DA, E, H1, H2, has_b2, has_b3, has_g1, has_g2)
    nc = _get_nc(key, *key)

    ident_np = np.eye(128, dtype=np.float32)
    ones_np = np.ones((128, 128), np.float32)
    phi2 = np.ascontiguousarray(phi.reshape(phi.shape[0], -1))
    w1aug = np.ascontiguousarray(
        np.concatenate([W1, b1[:, None, :]], axis=1)
    )  # [E, D+1, H1]
    w2c = np.ascontiguousarray(W2)
    w3c = np.ascontiguousarray(W3)

    in_maps = []
    for i in range(NCORES):
        m = {
            "z": np.ascontiguousarray(z[i * BC : (i + 1) * BC]),
            "a": np.ascontiguousarray(a[i * BC : (i + 1) * BC]),
            "zT": np.ascontiguousarray(
                z[i * BC : (i + 1) * BC].transpose(0, 2, 1)
            ),
            "aT": np.ascontiguousarray(
                a[i * BC : (i + 1) * BC].transpose(0, 2, 1)
            ),
            "phi": phi2,
            "w1": w1aug,
            "w2": w2c,
            "w3": w3c,
            "ident_d": ident_np,
            "ones_d": ones_np,
        }
        if has_b2:
            m["b2"] = np.ascontiguousarray(b2)
        if has_b3:
            m["b3"] = np.ascontiguousarray(b3)
        if has_g1:
            m["g1r"] = np.ascontiguousarray(np.repeat(g1, BC, axis=0))
            m["be1r"] = np.ascontiguousarray(np.repeat(be1, BC, axis=0))
        if has_g2:
            m["g2r"] = np.ascontiguousarray(np.repeat(g2, BC, axis=0))
            m["be2r"] = np.ascontiguousarray(np.repeat(be2, BC, axis=0))
        in_maps.append(m)
    return nc, in_maps


def kernel(**inputs):
    nc, in_maps = _prepare(**inputs)

    from concourse.bass_utils import run_bass_kernel_spmd

    res = run_bass_kernel_spmd(nc, in_maps, list(range(NCORES)))
    return np.concatenate([r["out"] for r in res.results], axis=0)
